# revision 1
# baseline (speedup 1.0000x reference)
"""EnhancedGAT kernel for 8 Trainium2 NeuronCores.

Strategy (v2): all five GAT message-passing layers run on-device as Bass
kernels composed inside one jax.jit/shard_map program. Nodes are sharded by
contiguous ranges (6250/core); each layer is transform (matmul) -> XLA
all_gather of the transformed features -> padded-degree indirect-DMA gather +
segment softmax + weighted aggregation. Graph pooling happens on-device via
an indicator matmul; the small MLP/conv head runs on host.

Indirect gathers only ever read ExternalInput DRAM tensors: intra-kernel
DRAM write -> indirect-read ordering is unreliable on this stack, so every
gather table crosses a kernel boundary (XLA sequences custom calls).
Padding entries point at an explicit zero row block in the gathered table.
"""

import numpy as np

N = 50000; E = 800000; G = 256; L = 1000
H = 4; C = 32; FD = 78; ED = 128; VOC = 26; K = 8
LOUT = L - K + 1; XTF = C * LOUT
HC = H * C

NCORES = 8
SH_REAL = N // NCORES          # 6250 real nodes per shard
SH_CMP = 6272                  # 49 tiles of 128 (compute rows)
SH = 6400                      # + 128-row zero block
TILES = SH_CMP // 128          # 49
NTAB = NCORES * SH             # 51200 rows in the all_gathered table
ZID = SH_CMP                   # a guaranteed-zero row (core 0's zero block)
DM = 36                        # padded max degree (actual max is 34)

_CACHE = {}


def _np(x):
    return np.asarray(x)


def _gid(n):
    """real node id -> padded table row id"""
    return (n // SH_REAL) * SH + (n % SH_REAL)


# ---------------------------------------------------------------- device ----

def _build_device_fn():
    import jax
    from jax.sharding import Mesh, PartitionSpec as P
    from jax.experimental.shard_map import shard_map
    from concourse.bass2jax import bass_jit
    import concourse.tile as tile
    import concourse.mybir as mybir
    import concourse.bass as bass
    from concourse.bass import ds

    f32 = mybir.dt.float32
    i32 = mybir.dt.int32
    u8 = mybir.dt.uint8
    AT = mybir.AluOpType
    ACT = mybir.ActivationFunctionType
    AX = mybir.AxisListType
    HSB = 4   # rotating gather buffers

    @bass_jit(target_bir_lowering=True)
    def k_tr1(nc, xT, w1):
        # xT [FD, SH_CMP] own shard (feature-major), w1 [FD, HC]
        z = nc.dram_tensor("z", [SH, HC], f32, kind="ExternalOutput")
        with tile.TileContext(nc) as tc:
            with (
                tc.tile_pool(name="sbuf", bufs=3) as pool,
                tc.tile_pool(name="io", bufs=1) as io,
                tc.tile_pool(name="psum", bufs=2, space="PSUM") as pp,
            ):
                wt = io.tile([FD, HC], f32)
                nc.sync.dma_start(out=wt[:], in_=w1[:])
                zt = io.tile([128, HC], f32)
                nc.vector.memset(zt[:], 0.0)
                nc.sync.dma_start(out=z[SH_CMP:SH, :], in_=zt[:])
                with tc.For_i(0, TILES) as iv:
                    xt_t = pool.tile([FD, 128], f32, tag="xt")
                    nc.sync.dma_start(out=xt_t[:], in_=xT[:, ds(iv * 128, 128)])
                    ps = pp.tile([128, HC], f32, space="PSUM", tag="ps")
                    nc.tensor.matmul(ps[:], lhsT=xt_t[:], rhs=wt[:],
                                     start=True, stop=True)
                    zo = pool.tile([128, HC], f32, tag="zo")
                    nc.vector.tensor_copy(out=zo[:], in_=ps[:])
                    nc.sync.dma_start(out=z[ds(iv * 128, 128), :], in_=zo[:])
        return (z,)

    def _agg_loop(nc, tc, pool, io, pp, zf, zsh, idx, mask8, asrc_r, adst_r,
                  bias_r, per_tile_tail):
        """Aggregation over node tiles; calls per_tile_tail(iv, h_t) with the
        finished [128, HC] node-tile features inside the For_i body."""
        asrc_t = io.tile([128, HC], f32)
        nc.sync.dma_start(out=asrc_t[:], in_=asrc_r[:])
        adst_t = io.tile([128, HC], f32)
        nc.sync.dma_start(out=adst_t[:], in_=adst_r[:])
        bias_t = io.tile([128, HC], f32)
        nc.sync.dma_start(out=bias_t[:], in_=bias_r[:])

        with tc.For_i(0, TILES) as iv:
            idx_t = pool.tile([128, DM], i32, tag="idxt")
            nc.sync.dma_start(out=idx_t[:], in_=idx[ds(iv * 128, 128), :])
            mk8_t = pool.tile([128, DM], u8, tag="mk8t")
            nc.sync.dma_start(out=mk8_t[:], in_=mask8[ds(iv * 128, 128), :])
            mk_t = pool.tile([128, DM], f32, tag="mkt")
            nc.vector.tensor_copy(out=mk_t[:], in_=mk8_t[:])
            # WAR pre-gates: the indirect DMA's read of idx_t is not
            # dependency-tracked; seeding every rotating gather buffer with a
            # value read from idx_t orders each gather (a WAR on its buffer)
            # after this iteration's idx load.
            for _ in range(HSB):
                g = pool.tile([128, 128], f32, tag="hs")
                nc.vector.tensor_copy(out=g[:, 0:1], in_=idx_t[:, 0:1])
            zsh_t = pool.tile([128, HC], f32, tag="zsh")
            nc.sync.dma_start(out=zsh_t[:], in_=zsh[ds(iv * 128, 128), :])
            tta = pool.tile([128, HC], f32, tag="tta")
            nc.vector.tensor_tensor(out=tta[:], in0=zsh_t[:], in1=adst_t[:],
                                    op=AT.mult)
            adn = pool.tile([128, H], f32, tag="adn")
            nc.vector.tensor_reduce(
                out=adn[:], in_=tta[:].rearrange("p (h c) -> p h c", h=H),
                axis=AX.X, op=AT.add)

            den = pool.tile([128, H], f32, tag="den")
            nc.vector.memset(den[:], 0.0)
            agg = pool.tile([128, HC], f32, tag="agg")
            nc.vector.memset(agg[:], 0.0)
            for d in range(DM):
                hs = pool.tile([128, 128], f32, tag="hs")
                nc.gpsimd.indirect_dma_start(
                    out=hs[:], out_offset=None, in_=zf[:],
                    in_offset=bass.IndirectOffsetOnAxis(
                        ap=idx_t[:, d:d + 1], axis=0),
                    bounds_check=NTAB - 1, oob_is_err=False)
                # s[p,h] = sum_c hs * a_src
                tms = pool.tile([128, HC], f32, tag="tms")
                nc.vector.tensor_tensor(out=tms[:], in0=hs[:], in1=asrc_t[:],
                                        op=AT.mult)
                ex = pool.tile([128, H], f32, tag="ex")
                nc.vector.tensor_reduce(
                    out=ex[:], in_=tms[:].rearrange("p (h c) -> p h c", h=H),
                    axis=AX.X, op=AT.add)
                nc.vector.tensor_tensor(out=ex[:], in0=ex[:], in1=adn[:],
                                        op=AT.add)
                nc.vector.scalar_tensor_tensor(
                    out=ex[:], in0=ex[:], scalar=0.2, in1=ex[:],
                    op0=AT.mult, op1=AT.max)
                nc.scalar.activation(ex[:], ex[:], ACT.Exp)
                nc.vector.tensor_tensor(
                    out=ex[:], in0=ex[:],
                    in1=mk_t[:, d:d + 1].to_broadcast([128, H]),
                    op=AT.mult)
                nc.vector.tensor_tensor(out=den[:], in0=den[:], in1=ex[:],
                                        op=AT.add)
                wrow = pool.tile([128, HC], f32, tag="wrow")
                nc.vector.tensor_tensor(
                    out=wrow[:].rearrange("p (h c) -> p h c", h=H),
                    in0=hs[:].rearrange("p (h c) -> p h c", h=H),
                    in1=ex[:].unsqueeze(2).to_broadcast([128, H, C]),
                    op=AT.mult)
                nc.vector.tensor_tensor(out=agg[:], in0=agg[:], in1=wrow[:],
                                        op=AT.add)
            nc.vector.reciprocal(den[:], den[:])
            nc.vector.tensor_tensor(
                out=agg[:].rearrange("p (h c) -> p h c", h=H),
                in0=agg[:].rearrange("p (h c) -> p h c", h=H),
                in1=den[:].unsqueeze(2).to_broadcast([128, H, C]),
                op=AT.mult)
            nc.vector.tensor_tensor(out=agg[:], in0=agg[:], in1=bias_t[:],
                                    op=AT.add)
            h_t = pool.tile([128, HC], f32, tag="ht")
            nc.vector.tensor_scalar(out=h_t[:], in0=agg[:], scalar1=0.0,
                                    scalar2=None, op0=AT.max)
            per_tile_tail(iv, h_t)

    @bass_jit(target_bir_lowering=True)
    def k_agg(nc, zf, zsh, idx, mask8, asrc_r, adst_r, bias_r, wnext, ident):
        z = nc.dram_tensor("z", [SH, HC], f32, kind="ExternalOutput")
        with tile.TileContext(nc) as tc:
            with (
                tc.tile_pool(name="sbuf", bufs=3) as pool,
                tc.tile_pool(name="io", bufs=1) as io,
                tc.tile_pool(name="psum", bufs=2, space="PSUM") as pp,
            ):
                idt = io.tile([128, 128], f32)
                nc.sync.dma_start(out=idt[:], in_=ident[:])
                wt = io.tile([HC, HC], f32)
                nc.sync.dma_start(out=wt[:], in_=wnext[:])
                zt = io.tile([128, HC], f32)
                nc.vector.memset(zt[:], 0.0)
                nc.sync.dma_start(out=z[SH_CMP:SH, :], in_=zt[:])

                def tail(iv, h_t):
                    pst = pp.tile([128, 128], f32, space="PSUM", tag="pst")
                    nc.tensor.transpose(out=pst[:], in_=h_t[:], identity=idt[:])
                    hT = pool.tile([128, 128], f32, tag="hT")
                    nc.vector.tensor_copy(out=hT[:], in_=pst[:])
                    ps = pp.tile([128, HC], f32, space="PSUM", tag="ps")
                    nc.tensor.matmul(ps[:], lhsT=hT[:], rhs=wt[:],
                                     start=True, stop=True)
                    zo = pool.tile([128, HC], f32, tag="zo")
                    nc.vector.tensor_copy(out=zo[:], in_=ps[:])
                    nc.sync.dma_start(out=z[ds(iv * 128, 128), :], in_=zo[:])

                _agg_loop(nc, tc, pool, io, pp, zf, zsh, idx, mask8,
                          asrc_r, adst_r, bias_r, tail)
        return (z,)

    @bass_jit(target_bir_lowering=True)
    def k_agg_pool(nc, zf, zsh, idx, mask8, asrc_r, adst_r, bias_r, batch):
        pooled = nc.dram_tensor("pooled", [G, HC], f32, kind="ExternalOutput")
        with tile.TileContext(nc) as tc:
            with (
                tc.tile_pool(name="sbuf", bufs=3) as pool,
                tc.tile_pool(name="io", bufs=1) as io,
                tc.tile_pool(name="psum", bufs=2, space="PSUM") as pp,
            ):
                iot = io.tile([128, G], i32)
                nc.gpsimd.iota(iot[:], pattern=[[1, G]], base=0,
                               channel_multiplier=0)
                iof = io.tile([128, G], f32)
                nc.vector.tensor_copy(out=iof[:], in_=iot[:])
                pool0 = io.tile([128, HC], f32)
                nc.vector.memset(pool0[:], 0.0)
                pool1 = io.tile([128, HC], f32)
                nc.vector.memset(pool1[:], 0.0)

                def tail(iv, h_t):
                    bt_t = pool.tile([128, 1], i32, tag="btt")
                    nc.sync.dma_start(out=bt_t[:], in_=batch[ds(iv * 128, 128), :])
                    btf_t = pool.tile([128, 1], f32, tag="btf")
                    nc.vector.tensor_copy(out=btf_t[:], in_=bt_t[:])
                    ind = pool.tile([128, G], f32, tag="ind")
                    nc.vector.tensor_tensor(
                        out=ind[:],
                        in0=btf_t[:, 0:1].to_broadcast([128, G]),
                        in1=iof[:], op=AT.is_equal)
                    for half, acc in ((0, pool0), (1, pool1)):
                        psg = pp.tile([128, HC], f32, space="PSUM", tag=f"pg{half}")
                        nc.tensor.matmul(
                            psg[:], lhsT=ind[:, half * 128:(half + 1) * 128],
                            rhs=h_t[:], start=True, stop=True)
                        nc.vector.tensor_tensor(out=acc[:], in0=acc[:],
                                                in1=psg[:], op=AT.add)

                _agg_loop(nc, tc, pool, io, pp, zf, zsh, idx, mask8,
                          asrc_r, adst_r, bias_r, tail)
                nc.sync.dma_start(out=pooled[0:128, :], in_=pool0[:])
                nc.sync.dma_start(out=pooled[128:G, :], in_=pool1[:])
        return (pooled,)

    def per_core(xT, idx, mask8, batch, w1, ws, asrcs, adsts, biases, ident):
        (z,) = k_tr1(xT, w1)
        for l in range(5):
            zfull = jax.lax.all_gather(z, "core", axis=0, tiled=True)
            if l < 4:
                (z,) = k_agg(zfull, z, idx, mask8, asrcs[l], adsts[l],
                             biases[l], ws[l], ident)
            else:
                (pooled,) = k_agg_pool(zfull, z, idx, mask8, asrcs[l],
                                       adsts[l], biases[l], batch)
        return pooled

    devices = jax.devices()[:NCORES]
    mesh = Mesh(np.asarray(devices), ("core",))
    fn = jax.jit(shard_map(
        per_core, mesh=mesh,
        in_specs=(P("core"), P("core"), P("core"), P("core"),
                  P(), P(), P(), P(), P(), P()),
        out_specs=P("core"), check_rep=False))
    return fn


# ------------------------------------------------------------------ host ----

def _prep_graph(src, dst):
    """Build per-core padded adjacency [NCORES*SH_CMP, DM] of table row ids."""
    order = np.argsort(dst.astype(np.int32), kind="stable")
    src_s = src[order]
    dst_s = dst[order]
    counts = np.bincount(dst_s, minlength=N).astype(np.int64)
    dmax = int(counts.max())
    assert dmax <= DM, f"max degree {dmax} exceeds padded DM={DM}"
    indptr = np.zeros(N, np.int64)
    np.cumsum(counts[:-1], out=indptr[1:])
    rank = np.arange(len(dst_s), dtype=np.int64) - indptr[dst_s]

    idx = np.full((NCORES * SH_CMP, DM), ZID, np.int32)
    mask = np.zeros((NCORES * SH_CMP, DM), np.uint8)
    # dst row in padded layout
    dst_row = (dst_s // SH_REAL) * SH_CMP + (dst_s % SH_REAL)
    idx[dst_row, rank] = _gid(src_s).astype(np.int32)
    mask[dst_row, rank] = 1
    return idx, mask


def kernel(**inputs):
    x = _np(inputs["x"]).astype(np.float32)
    edge_index = _np(inputs["edge_index"]).astype(np.int64)
    batch = _np(inputs["batch"]).astype(np.int64)
    target = _np(inputs["target"]).astype(np.int64)

    loop = np.arange(N, dtype=np.int64)
    src = np.concatenate([edge_index[0], loop])
    dst = np.concatenate([edge_index[1], loop])
    idx, mask = _prep_graph(src, dst)

    # x shards, feature-major [NCORES*FD, SH_CMP]
    xT = np.zeros((NCORES, FD, SH_CMP), np.float32)
    for c in range(NCORES):
        xT[c, :, :SH_REAL] = x[c * SH_REAL:(c + 1) * SH_REAL].T
    xT = xT.reshape(NCORES * FD, SH_CMP)

    batch_pad = np.full((NCORES, SH_CMP, 1), -1, np.int32)
    for c in range(NCORES):
        batch_pad[c, :SH_REAL, 0] = batch[c * SH_REAL:(c + 1) * SH_REAL]
    batch_pad = batch_pad.reshape(NCORES * SH_CMP, 1)

    W1 = _np(inputs["W1"]).astype(np.float32)
    Ws = _np(inputs["Ws"]).astype(np.float32)
    att_src1 = _np(inputs["att_src1"]).astype(np.float32).reshape(1, HC)
    att_dst1 = _np(inputs["att_dst1"]).astype(np.float32).reshape(1, HC)
    att_srcs = _np(inputs["att_srcs"]).astype(np.float32).reshape(4, 1, HC)
    att_dsts = _np(inputs["att_dsts"]).astype(np.float32).reshape(4, 1, HC)
    bias1 = _np(inputs["bias1"]).astype(np.float32).reshape(1, HC)
    biases = _np(inputs["biases"]).astype(np.float32).reshape(4, 1, HC)

    rep = lambda a: np.tile(a, (128, 1)).astype(np.float32)        # [128, HC]
    asrcs = np.stack([rep(att_src1)] + [rep(att_srcs[i]) for i in range(4)])
    adsts = np.stack([rep(att_dst1)] + [rep(att_dsts[i]) for i in range(4)])
    biass = np.stack([rep(bias1)] + [rep(biases[i]) for i in range(4)])
    ws = np.stack([Ws[i] for i in range(4)])                        # [4, HC, HC]
    ident = np.eye(128, dtype=np.float32)

    fn = _CACHE.get("fn")
    if fn is None:
        fn = _build_device_fn()
        _CACHE["fn"] = fn

    pooled_parts = np.asarray(fn(xT, idx, mask, batch_pad,
                                 W1, ws, asrcs, adsts, biass, ident))
    pooled = pooled_parts.reshape(NCORES, G, HC).sum(axis=0)

    # ---------------- host head ----------------
    fc_xd_w = _np(inputs["fc_xd_w"]).astype(np.float32)
    fc_xd_b = _np(inputs["fc_xd_b"]).astype(np.float32)
    xd = np.maximum(pooled @ fc_xd_w + fc_xd_b, 0.0)

    emb = _np(inputs["emb"]).astype(np.float32)
    conv_w = _np(inputs["conv_w"]).astype(np.float32)
    conv_b = _np(inputs["conv_b"]).astype(np.float32)
    M = np.einsum("vd,cdk->vck", emb, conv_w)      # [VOC, C, K]
    cv = np.zeros((G, C, LOUT), np.float32)
    for k in range(K):
        cv += M[:, :, k][target[:, k:k + LOUT]].transpose(0, 2, 1)
    cv += conv_b[None, :, None]
    fc_xt_w = _np(inputs["fc_xt_w"]).astype(np.float32)
    fc_xt_b = _np(inputs["fc_xt_b"]).astype(np.float32)
    xt = np.maximum(cv.reshape(G, -1) @ fc_xt_w + fc_xt_b, 0.0)

    xc = np.concatenate([xd, xt], axis=1)          # [G, 256]
    fc1_w = _np(inputs["fc1_w"]).astype(np.float32)
    fc1_b = _np(inputs["fc1_b"]).astype(np.float32)
    fc2_w = _np(inputs["fc2_w"]).astype(np.float32)
    fc2_b = _np(inputs["fc2_b"]).astype(np.float32)
    out_w = _np(inputs["out_w"]).astype(np.float32)
    out_b = _np(inputs["out_b"]).astype(np.float32)
    h1 = np.maximum(xc @ fc1_w + fc1_b, 0.0)
    h2 = np.maximum(h1 @ fc2_w + fc2_b, 0.0)
    return (h2 @ out_w + out_b).astype(np.float32)

